# revision 55
# baseline (speedup 1.0000x reference)
"""Self-contained Trainium2 Bass kernel for nn_GCN3 (3-layer GCN + BN + final linear).

Strategy: nodes/edges sharded by destination across 8 NeuronCores; edges
pre-sorted by destination and packed into 128-edge tiles. The full GCN
symmetric normalization (deg^-1/2 on both ends times edge weight) is baked
into a per-edge bf16 weight on the host. On device, per-tile gathers pull
source features from a bf16 node-feature table exchanged via AllGather;
destination one-hot matrices are built on the fly with a batched is_equal
against an iota, and a single matmul per tile accumulates [H, 128] block
outputs in PSUM. BatchNorm is folded into the next GEMM; BN statistics are
AllReduced. The compiled program and jitted runner are cached across calls.
"""
import sys
import numpy as np
import ml_dtypes

for _p in ("/opt/trn_rl_repo",):
    if _p not in sys.path:
        sys.path.insert(0, _p)

P = 128          # partitions / edges per tile
F_IN = 64
H = 32
C_OUT = 2
BN_EPS = 1e-5
N_CORES = 8
TBG = 32         # tiles per gather buffer
TBO = 16         # tiles per one-hot build
FCHUNK = 512     # final linear chunk

BF16 = ml_dtypes.bfloat16


def _to_bf16(a_f32):
    """Round-to-nearest-even f32 -> bf16 via integer ops (faster than astype)."""
    u = np.ascontiguousarray(a_f32).view(np.uint32)
    u16 = ((u + 0x7FFF + ((u >> 16) & 1)) >> 16).astype(np.uint16)
    return u16.view(BF16)


try:
    import numba as _nb

    @_nb.njit(cache=True, fastmath=False)
    def _edges_nb(row, col, w, dis, nxt, brow, tbase, gwv, dhiv):
        # per edge: counting-rank slot assignment + norm (deg^-1/2 w deg^-1/2)
        # + bf16 round-to-nearest-even + value packing + both scatters
        scr = np.empty(1, np.float32)
        scr_u = scr.view(np.uint32)
        for e in range(row.shape[0]):
            c = col[e]
            s_node = row[e]
            g = c >> 7
            r = nxt[g]
            nxt[g] = r + 1
            ri = brow[g] + (r & 127)
            ci = tbase[g] + (r >> 7)
            scr[0] = dis[s_node] * w[e] * dis[c]
            u = scr_u[0]
            b16 = (u + np.uint32(0x7FFF) + ((u >> np.uint32(16))
                                            & np.uint32(1))) >> np.uint32(16)
            gwv[ri, ci] = (b16 << np.uint32(16)) | (np.uint32(s_node)
                                                    & np.uint32(0xFFFF))
            dhiv[ri, ci] = np.uint8(((c & 127) << 1) | (s_node >> 16))

    @_nb.njit(cache=True, fastmath=False)
    def _xc_nb(xu16, xc, n_cores, SH, SH0, SH1, N):
        # truncating f32->bf16 transpose of node columns [SH0, SH1) of
        # every core into concat layout [n_cores*F, SH1-SH0]
        F = xu16.shape[1] // 2
        for c in range(n_cores):
            base_row = c * F
            n1 = min(c * SH + SH1, N)
            for n in range(c * SH + SH0, n1):
                cloc = n - c * SH - SH0
                for f in range(F):
                    xc[base_row + f, cloc] = xu16[n, 2 * f + 1]
except ImportError:          # pragma: no cover
    _nb = None


def preprocess(x, edge_index, edge_weights, n_cores=N_CORES, put=None):
    """Vectorized host-side tiling. Returns (meta, arrays).

    If `put` is given, each big array is handed to it as soon as it is
    built (used to overlap host->device transfer with preprocessing);
    arrays dict then holds whatever `put` returned.
    """
    N = x.shape[0]
    SH = -(-N // (n_cores * P)) * P               # nodes per core (padded)
    NPAD = SH * n_cores
    NBLK = SH // P
    out = {}
    keep = put if put is not None else (lambda name, a: a)

    # x first -> concat-layout bf16 (truncating cast: BN downstream
    # cancels the tiny systematic shrink), in two column-halves so the
    # first transfer hits the wire after only half the build
    xu16 = np.ascontiguousarray(np.asarray(x, np.float32)).view(np.uint16)
    SHH = (NBLK // 2) * P
    for name, s0, s1 in (("xTa", 0, SHH), ("xTb", SHH, SH)):
        xch = np.zeros((n_cores * F_IN, s1 - s0), np.uint16)
        if _nb is not None:
            _xc_nb(xu16, xch, n_cores, SH, s0, s1, N)
        else:
            for c in range(n_cores):
                lo = c * SH + s0
                hi = min(c * SH + s1, N)
                if hi > lo:
                    xch[c * F_IN:(c + 1) * F_IN, :hi - lo] = \
                        xu16[lo:hi, 1::2].T
        out[name] = keep(name, xch.view(BF16))

    row = np.asarray(edge_index[0], np.int32)
    col = np.asarray(edge_index[1], np.int32)
    w = np.asarray(edge_weights, np.float32)
    E = len(col)

    # symmetric normalization baked into per-edge weights
    deg = np.bincount(col, weights=w, minlength=N) + 1.0   # + self-loop
    dis = (1.0 / np.sqrt(deg)).astype(np.float32)

    NGB = NPAD // P
    # per-block self-loop count: loops of block g occupy ranks [0, nloop_g)
    nloop = np.minimum(np.maximum(N - np.arange(NGB) * P, 0), P)
    cnt = np.bincount(col >> 7, minlength=NGB) + nloop
    tiles_blk = np.maximum(-(-cnt.reshape(n_cores, NBLK) // P), 1).max(axis=0)
    NT = int(tiles_blk.sum())
    tile_off = np.zeros(NBLK + 1, np.int64)
    tile_off[1:] = np.cumsum(tiles_blk)

    base_g = ((np.arange(NGB, dtype=np.int32) // NBLK) * P * NT
              + np.tile(tile_off[:NBLK], n_cores)).astype(np.int32)

    brow_g = ((np.arange(NGB, dtype=np.int32) // NBLK) * P).astype(np.int32)
    tbase_g = np.tile(tile_off[:NBLK], n_cores).astype(np.int32)
    # one merged buffer: per partition-row, 4*NT bytes of gw then NT of dhi
    #   gw = low16(src) | bf16(norm) << 16 ; dhi = dloc << 1 | src >> 16
    buf = np.zeros((n_cores * P, 5 * NT), np.uint8)
    gwv = buf[:, :4 * NT].view(np.uint32)          # [n_cores*P, NT]
    dhiv = buf[:, 4 * NT:]                         # [n_cores*P, NT]
    # edge slots: counting ranks per block; self-loops occupy the first
    # nloop_g ranks
    if _nb is not None:
        _edges_nb(row, col, w, dis, nloop.astype(np.int32), brow_g,
                  tbase_g, gwv, dhiv)
    else:
        order = np.argsort((col >> 7).astype(np.int16), kind="stable")
        ch = cnt - nloop
        bstart = np.zeros(NGB, np.int32)
        np.cumsum(ch[:-1], out=bstart[1:], dtype=np.int32)
        r = np.arange(E, dtype=np.int32)
        r -= np.repeat((bstart - nloop).astype(np.int32), ch)
        ri = np.repeat(brow_g, ch) + (r & 127)
        ci = np.repeat(tbase_g, ch) + (r >> 7)
        rs = np.empty(E, np.int32); rs[order] = ri
        cs = np.empty(E, np.int32); cs[order] = ci
        norm_bf = _to_bf16(dis[row] * w * dis[col])
        val32 = norm_bf.view(np.uint16).astype(np.uint32)
        val32 <<= 16
        val32 |= (row & np.int32(0xFFFF)).astype(np.uint32)
        gwv[rs, cs] = val32
        dhiv[rs, cs] = ((col & 127) << 1 | (row >> 16)).astype(np.uint8)

    # self-loop slots: node v -> block g=v>>7, rank v&127 -> tile 0 of g
    v = np.arange(N, dtype=np.int32)
    gv = v >> 7
    rl = brow_g[gv] + (v & 127)
    cl = tbase_g[gv]
    lw = _to_bf16(dis * dis).view(np.uint16).astype(np.uint32) << 16
    lw |= v.astype(np.uint32) & 0xFFFF
    gwv[rl, cl] = lw
    dhiv[rl, cl] = ((v & 127) << 1 | (v >> 16)).astype(np.uint8)
    out["gwd"] = keep("gwd", buf)

    meta = dict(N=N, NPAD=NPAD, SH=SH, NBLK=NBLK, NT=NT,
                tiles_blk=tuple(int(t) for t in tiles_blk),
                tile_off=tile_off, n_cores=n_cores)
    return meta, out


import concourse.bass as bass
import concourse.bacc as bacc
import concourse.mybir as mybir
import concourse.tile as tile

F32 = mybir.dt.float32
MBF16 = mybir.dt.bfloat16
I32 = mybir.dt.int32
U8 = mybir.dt.uint8
AF = mybir.ActivationFunctionType


def build_program(meta):
    N = meta["N"]; NPAD = meta["NPAD"]; SH = meta["SH"]; NBLK = meta["NBLK"]
    NT = meta["NT"]
    tiles_blk = meta["tiles_blk"]; tile_off = meta["tile_off"]
    n_cores = meta["n_cores"]

    nc = bacc.Bacc()

    NBA = NBLK // 2
    SHH = NBA * P
    xTa_in = nc.declare_dram_parameter("xTa", [F_IN, SHH], MBF16,
                                       isOutput=False)
    xTb_in = nc.declare_dram_parameter("xTb", [F_IN, SH - SHH], MBF16,
                                       isOutput=False)
    gwd_in = nc.declare_dram_parameter("gwd", [P, 5 * NT], U8, isOutput=False)
    # output is AllGathered on-device so the host fetches ONE replica
    # (one D2H round-trip) instead of 8 per-core shards
    w1_in = nc.declare_dram_parameter("w1", [F_IN, H], MBF16, isOutput=False)
    cpack_in = nc.declare_dram_parameter("cpack", [H, 86], F32, isOutput=False)
    iota_in = nc.declare_dram_parameter("iota128", [P, P], MBF16, isOutput=False)
    ones_in = nc.declare_dram_parameter("ones_row", [1, SH], MBF16, isOutput=False)
    out_par = nc.declare_dram_parameter("out", [n_cores * C_OUT, SH], F32,
                                        isOutput=True)

    rg = [list(range(n_cores))]

    with tile.TileContext(nc) as tc:
        with (
            tc.tile_pool(name="cst", bufs=1) as cst,
            tc.tile_pool(name="big", bufs=1) as big,
            tc.tile_pool(name="st", bufs=2) as st,
            tc.tile_pool(name="gap", bufs=3) as gap,
            tc.tile_pool(name="ohp", bufs=3) as ohp,
            tc.tile_pool(name="wk", bufs=2) as wk,
            tc.tile_pool(name="psA", bufs=2, space="PSUM") as psA,
            tc.tile_pool(name="psB", bufs=4, space="PSUM") as psB,
            tc.tile_pool(name="dr", bufs=1, space="DRAM") as dr,
        ):
            # ---- consts ----
            w1_sb = cst.tile([F_IN, H], MBF16); nc.sync.dma_start(w1_sb[:], w1_in[:])
            cpack_sb = cst.tile([H, 86], F32)
            nc.sync.dma_start(cpack_sb[:], cpack_in[:])
            w23_sb = cpack_sb[:, 0:64]
            wl_sb = cpack_sb[:, 64:70]
            bl_sb = cpack_sb[0:C_OUT, 70:71]
            vec_sb = cpack_sb[:, 71:80]
            scorr_sb = cpack_sb[:, 80:86]
            iota_sb = cst.tile([P, P], MBF16); nc.sync.dma_start(iota_sb[:], iota_in[:])
            gw_sb = cst.tile([P, 4 * NT], U8)
            nc.sync.dma_start(gw_sb[:], gwd_in[:, 0:4 * NT])
            gw16 = gw_sb[:].bitcast(mybir.dt.uint16)       # [P, 2*NT]
            glo_ap = gw16[:, 0:2 * NT:2]                   # low halves
            dhi_t = cst.tile([P, NT], U8)
            nc.sync.dma_start(dhi_t[:], gwd_in[:, 4 * NT:5 * NT])
            dhi_u8 = dhi_t[:]
            wgt_sb = cst.tile([P, NT], MBF16)
            nc.vector.tensor_copy(wgt_sb[:], gw16[:, 1:2 * NT:2].bitcast(MBF16))
            # unpack: dloc = dhi >> 1 ; gidx = ((dhi & 1) << 16) + glo
            dhi_i = cst.tile([P, NT], I32)
            nc.vector.tensor_copy(dhi_i[:], dhi_u8)
            dloc_i = cst.tile([P, NT], I32)
            nc.vector.tensor_scalar(
                out=dloc_i[:], in0=dhi_i[:], scalar1=1, scalar2=None,
                op0=mybir.AluOpType.logical_shift_right)
            dloc_sb = cst.tile([P, NT], MBF16)
            nc.vector.tensor_copy(dloc_sb[:], dloc_i[:])
            gidx_sb = cst.tile([P, NT], I32)
            # hi<<16 = (dhi - (dloc<<1)) << 16 = (dhi<<16) - (dloc<<17)
            nc.vector.tensor_scalar(
                out=gidx_sb[:], in0=dhi_i[:], scalar1=16, scalar2=None,
                op0=mybir.AluOpType.logical_shift_left)
            nc.vector.tensor_scalar(
                out=dhi_i[:], in0=dloc_i[:], scalar1=17, scalar2=None,
                op0=mybir.AluOpType.logical_shift_left)
            nc.vector.tensor_tensor(out=gidx_sb[:], in0=gidx_sb[:], in1=dhi_i[:],
                                    op=mybir.AluOpType.subtract)
            glo_i = cst.tile([P, NT], I32)
            nc.vector.tensor_copy(glo_i[:], glo_ap)
            nc.vector.tensor_tensor(out=gidx_sb[:], in0=gidx_sb[:], in1=glo_i[:],
                                    op=mybir.AluOpType.add)

            # ---- slabs (relu outputs + ones row) ----
            slabs = []
            for k in range(3):
                s = big.tile([H + 1, SH], MBF16, tag=f"slab{k}")
                nc.sync.dma_start(s[H:H + 1, :], ones_in[:])
                slabs.append(s)
            hprime = big.tile([P, NBLK, H], MBF16, tag="hprime")

            oout_t = dr.tile([C_OUT, SH], F32, tag="oout")
            ofull_t = dr.tile([n_cores * C_OUT, SH], F32, tag="ofull")
            own_t = dr.tile([SH, H], MBF16, tag="own")
            table_t = dr.tile([NPAD, H], MBF16, tag="table")
            stat_in_t = dr.tile([H, 2], F32, tag="stat_in")
            stat_out_t = dr.tile([H, 2], F32, tag="stat_out")

            s_tiles, t_tiles = [], []

            for L in range(3):
                bvec = vec_sb[:, L:L + 1]
                gvec = vec_sb[:, 3 + L:4 + L]
                bevec = vec_sb[:, 6 + L:7 + L]

                # ---- GEMM -> h' (bf16) ----
                if L == 0:
                    for b in range(NBLK):
                        xblk = wk.tile([F_IN, P], MBF16, tag="xblk")
                        if b < NBA:
                            xsrc = xTa_in[:, b * P:(b + 1) * P]
                        else:
                            xsrc = xTb_in[:, (b - NBA) * P:(b - NBA + 1) * P]
                        nc.sync.dma_start(xblk[:], xsrc)
                        h_ps = psA.tile([P, H], F32, space="PSUM", tag="a")
                        nc.tensor.matmul(out=h_ps[:], lhsT=xblk[:], rhs=w1_sb[:],
                                         start=True, stop=True)
                        nc.vector.tensor_copy(hprime[:, b, :], h_ps[:])
                else:
                    s_prev, t_prev = s_tiles[-1], t_tiles[-1]
                    wsl = w23_sb[:, (L - 1) * H:L * H]
                    w_ext = wk.tile([H + 1, H], MBF16, tag="wext")
                    nc.vector.tensor_scalar_mul(w_ext[0:H, :], wsl, s_prev[:, :1])
                    br_ps = psB.tile([1, H], F32, space="PSUM", tag="b")
                    nc.tensor.matmul(out=br_ps[:], lhsT=t_prev[:], rhs=wsl,
                                     start=True, stop=True)
                    nc.vector.tensor_copy(w_ext[H:H + 1, :], br_ps[:])
                    for b in range(NBLK):
                        h_ps = psA.tile([P, H], F32, space="PSUM", tag="a")
                        nc.tensor.matmul(
                            out=h_ps[:], lhsT=slabs[L - 1][:, b * P:(b + 1) * P],
                            rhs=w_ext[:], start=True, stop=True)
                        nc.vector.tensor_copy(hprime[:, b, :], h_ps[:])

                # ---- exchange ----
                nc.sync.dma_start(
                    own_t.opt().rearrange("(b p) h -> p b h", p=P), hprime[:])
                nc.gpsimd.collective_compute(
                    "AllGather", mybir.AluOpType.bypass,
                    ins=[own_t.opt()], outs=[table_t.opt()], replica_groups=rg)

                # ---- aggregate ----
                stats_s = st.tile([H, NBLK], F32, tag="ss")
                stats_q = st.tile([H, NBLK], F32, tag="sq")
                sq_scr = st.tile([H, P], F32, tag="sqscr")
                cur_gb = cur_ob = -1
                gath = oh = None
                for b in range(NBLK):
                    T = int(tiles_blk[b])
                    out_ps = psB.tile([H, P], F32, space="PSUM", tag="b")
                    for tl in range(T):
                        t = int(tile_off[b]) + tl
                        gb_i, gb_s = divmod(t, TBG)
                        if gb_i != cur_gb:
                            cur_gb = gb_i
                            t0 = gb_i * TBG
                            tn = min(TBG, NT - t0)
                            gath = gap.tile([P, TBG, H], MBF16, tag="ga")
                            for tg in range(tn):
                                nc.gpsimd.indirect_dma_start(
                                    out=gath[:, tg, :], out_offset=None,
                                    in_=table_t.opt(),
                                    in_offset=bass.IndirectOffsetOnAxis(
                                        ap=gidx_sb[:, t0 + tg:t0 + tg + 1], axis=0))
                            nc.vector.tensor_tensor(
                                out=gath[:, :tn, :], in0=gath[:, :tn, :],
                                in1=wgt_sb[:, t0:t0 + tn].unsqueeze(2)
                                    .to_broadcast([P, tn, H]),
                                op=mybir.AluOpType.mult)
                        ob_i, ob_s = divmod(t, TBO)
                        if ob_i != cur_ob:
                            cur_ob = ob_i
                            o0 = ob_i * TBO
                            on = min(TBO, NT - o0)
                            oh = ohp.tile([P, TBO, P], MBF16, tag="oh")
                            nc.vector.tensor_tensor(
                                out=oh[:, :on, :],
                                in0=dloc_sb[:, o0:o0 + on].unsqueeze(2)
                                    .to_broadcast([P, on, P]),
                                in1=iota_sb[:].unsqueeze(1).to_broadcast([P, on, P]),
                                op=mybir.AluOpType.is_equal)
                        nc.tensor.matmul(
                            out=out_ps[:], lhsT=gath[:, gb_s, :], rhs=oh[:, ob_s, :],
                            start=(tl == 0), stop=(tl == T - 1))
                    # epilogue: bias, relu, stats
                    dst = slabs[L][0:H, b * P:(b + 1) * P]
                    nc.scalar.activation(dst, out_ps[:], AF.Relu, bias=bvec)
                    nc.vector.tensor_reduce(out=stats_s[:, b:b + 1], in_=dst,
                                            axis=mybir.AxisListType.X,
                                            op=mybir.AluOpType.add)
                    nc.scalar.activation(sq_scr[:], dst, AF.Square,
                                         accum_out=stats_q[:, b:b + 1])

                # ---- BN stats -> s, t ----
                st2 = st.tile([H, 2], F32, tag="st2")
                nc.vector.tensor_reduce(out=st2[:, 0:1], in_=stats_s[:],
                                        axis=mybir.AxisListType.X,
                                        op=mybir.AluOpType.add)
                nc.vector.tensor_reduce(out=st2[:, 1:2], in_=stats_q[:],
                                        axis=mybir.AxisListType.X,
                                        op=mybir.AluOpType.add)
                nc.sync.dma_start(stat_in_t[:], st2[:])
                nc.gpsimd.collective_compute(
                    "AllReduce", mybir.AluOpType.add,
                    ins=[stat_in_t.opt()], outs=[stat_out_t.opt()], replica_groups=rg)
                stg = st.tile([H, 2], F32, tag="stg")
                nc.sync.dma_start(stg[:], stat_out_t.opt())
                nc.vector.tensor_tensor(out=stg[:], in0=stg[:],
                                        in1=scorr_sb[:, 2 * L:2 * L + 2],
                                        op=mybir.AluOpType.subtract)
                nc.vector.tensor_scalar_mul(stg[:], stg[:], 1.0 / N)
                mu = stg[:, 0:1]
                s_t = st.tile([H, 1], F32, tag=f"s{L}")
                t_t = st.tile([H, 1], F32, tag=f"t{L}")
                var_t = st.tile([H, 1], F32, tag="var")
                nc.vector.tensor_tensor(out=var_t[:], in0=mu, in1=mu,
                                        op=mybir.AluOpType.mult)
                nc.vector.tensor_tensor(out=var_t[:], in0=stg[:, 1:2], in1=var_t[:],
                                        op=mybir.AluOpType.subtract)
                nc.vector.tensor_scalar_add(var_t[:], var_t[:], BN_EPS)
                nc.scalar.activation(var_t[:], var_t[:], AF.Sqrt)
                nc.vector.reciprocal(var_t[:], var_t[:])
                nc.vector.tensor_tensor(out=s_t[:], in0=gvec, in1=var_t[:],
                                        op=mybir.AluOpType.mult)
                nc.vector.tensor_tensor(out=t_t[:], in0=mu, in1=s_t[:],
                                        op=mybir.AluOpType.mult)
                nc.vector.tensor_tensor(out=t_t[:], in0=bevec, in1=t_t[:],
                                        op=mybir.AluOpType.subtract)
                s_tiles.append(s_t)
                t_tiles.append(t_t)

            # ---- final linear ----
            c2_ps = psB.tile([C_OUT, 1], F32, space="PSUM", tag="b")
            for k in range(3):
                nc.tensor.matmul(out=c2_ps[:], lhsT=wl_sb[:, 2 * k:2 * k + 2],
                                 rhs=t_tiles[k][:], start=(k == 0), stop=(k == 2))
            c2_sb = st.tile([C_OUT, 1], F32, tag="c2sb")
            nc.vector.tensor_tensor(out=c2_sb[:], in0=c2_ps[:], in1=bl_sb[:],
                                    op=mybir.AluOpType.add)
            wls = []
            for k in range(3):
                wsc = st.tile([H, C_OUT], MBF16, tag=f"wls{k}")
                nc.vector.tensor_scalar_mul(wsc[:], wl_sb[:, 2 * k:2 * k + 2],
                                            s_tiles[k][:, :1])
                wls.append(wsc)
            for ch0 in range(0, SH, FCHUNK):
                cw = min(FCHUNK, SH - ch0)
                f_ps = psB.tile([C_OUT, FCHUNK], F32, space="PSUM", tag="b")
                for k in range(3):
                    nc.tensor.matmul(out=f_ps[:, :cw], lhsT=wls[k][:],
                                     rhs=slabs[k][0:H, ch0:ch0 + cw],
                                     start=(k == 0), stop=(k == 2))
                f_sb = wk.tile([C_OUT, FCHUNK], F32, tag="fsb")
                nc.scalar.activation(f_sb[:, :cw], f_ps[:, :cw], AF.Identity,
                                     bias=c2_sb[:, :1])
                nc.sync.dma_start(oout_t.opt()[:, ch0:ch0 + cw], f_sb[:, :cw])
            nc.gpsimd.collective_compute(
                "AllGather", mybir.AluOpType.bypass,
                ins=[oout_t.opt()], outs=[ofull_t.opt()], replica_groups=rg)
            nc.sync.dma_start(out_par[:], ofull_t.opt())
    nc.compile()
    return nc


def make_weight_inputs(meta, weights, xscale=None):
    """Core-uniform constant arrays. W1 rows absorb the int8 x scales."""
    w1 = np.asarray(weights["W1"], np.float32)
    if xscale is not None:
        w1 = w1 * xscale[:, None]
    n_pad = meta["NPAD"] - meta["N"]
    b_relu = [np.maximum(np.asarray(weights[f"b{k}"], np.float32), 0.0)
              for k in (1, 2, 3)]
    vec = np.stack([np.asarray(weights[k], np.float32) for k in
                    ("b1", "b2", "b3", "g1", "g2", "g3", "be1", "be2", "be3")],
                   axis=1)
    cpack = np.zeros((H, 86), np.float32)
    cpack[:, 0:32] = np.asarray(weights["W2"], np.float32)
    cpack[:, 32:64] = np.asarray(weights["W3"], np.float32)
    cpack[:, 64:70] = (np.asarray(weights["Wl"], np.float32)
                       .reshape(3, H, C_OUT).transpose(1, 0, 2)
                       .reshape(H, 3 * C_OUT))
    cpack[0:C_OUT, 70] = np.asarray(weights["bl"], np.float32)
    cpack[:, 71:80] = vec
    cpack[:, 80:86] = np.concatenate(
        [np.stack([n_pad * br, n_pad * br ** 2], axis=1) for br in b_relu],
        axis=1)
    return {
        "w1": w1.astype(BF16),
        "cpack": cpack,
    }


# -------- cached compile + jitted runner --------
_CACHE = {}


def _get_compiled(meta):
    key = (meta["N"], meta["NT"], meta["tiles_blk"])
    hit = _CACHE.get(key)
    if hit is not None:
        return hit

    nc = build_program(meta)

    import jax
    from jax.sharding import Mesh, PartitionSpec
    from jax.experimental.shard_map import shard_map
    from concourse import bass2jax

    bass2jax.install_neuronx_cc_hook()
    partition_name = (nc.partition_id_tensor.name
                      if nc.partition_id_tensor else None)
    in_names, out_names, out_avals, zero_shapes = [], [], [], []
    for alloc in nc.m.functions[0].allocations:
        if not isinstance(alloc, mybir.MemoryLocationSet):
            continue
        name = alloc.memorylocations[0].name
        if alloc.kind == "ExternalInput":
            if name != partition_name:
                in_names.append(name)
        elif alloc.kind == "ExternalOutput":
            shape = tuple(alloc.tensor_shape)
            dtype = mybir.dt.np(alloc.dtype)
            out_names.append(name)
            out_avals.append(jax.core.ShapedArray(shape, dtype))
            zero_shapes.append((shape, dtype))
    n_params = len(in_names)
    all_names = list(in_names) + list(out_names)
    if partition_name is not None:
        all_names.append(partition_name)
    donate = tuple(range(n_params, n_params + len(out_names)))

    def _body(*args):
        operands = list(args)
        if partition_name is not None:
            operands.append(bass2jax.partition_id_tensor())
        outs = bass2jax._bass_exec_p.bind(
            *operands,
            out_avals=tuple(out_avals),
            in_names=tuple(all_names),
            out_names=tuple(out_names),
            lowering_input_output_aliases=(),
            sim_require_finite=False,
            sim_require_nnan=False,
            nc=nc,
        )
        return tuple(outs)

    devices = jax.devices()[:meta["n_cores"]]
    mesh = Mesh(np.asarray(devices), ("core",))
    in_specs = (PartitionSpec("core"),) * (n_params + len(out_names))
    out_specs = (PartitionSpec("core"),) * len(out_names)
    sharded = jax.jit(
        shard_map(_body, mesh=mesh, in_specs=in_specs, out_specs=out_specs,
                  check_rep=False),
        donate_argnums=donate, keep_unused=True)
    entry = (sharded, in_names, out_names, zero_shapes)
    _CACHE[key] = entry
    return entry


def kernel(**inputs):
    import jax
    from jax.sharding import Mesh, NamedSharding, PartitionSpec

    x = inputs["x"]
    edge_index = inputs["edge_index"]
    edge_weights = inputs["edge_weights"]
    weights = {k: np.asarray(inputs[k], np.float32) for k in (
        "W1", "b1", "g1", "be1", "W2", "b2", "g2", "be2",
        "W3", "b3", "g3", "be3", "Wl", "bl")}

    mesh = Mesh(np.asarray(jax.devices()[:N_CORES]), ("core",))
    sh = NamedSharding(mesh, PartitionSpec("core"))
    put = lambda name, a: jax.device_put(a, sh)

    # output zero-buffer (donated) made on device, and static consts; both
    # are known from shapes alone and overlap everything else
    SH = -(-x.shape[0] // (N_CORES * P)) * P
    zmk = _CACHE.get(("zmk", SH))
    if zmk is None:
        import jax.numpy as jnp
        zmk = jax.jit(
            lambda: jnp.zeros((N_CORES * N_CORES * C_OUT, SH), jnp.float32),
            out_shardings=sh)
        _CACHE[("zmk", SH)] = zmk
    zeros_out = zmk()
    static = _CACHE.get(("static", SH))
    if static is None:
        iota = np.tile(np.arange(P, dtype=np.float32), (P, 1)).astype(BF16)
        static = {
            "iota128": put("iota128", np.ascontiguousarray(
                np.broadcast_to(iota, (N_CORES,) + iota.shape)
                  .reshape(N_CORES * P, P))),
            "ones_row": put("ones_row", np.ones((N_CORES, SH), BF16)),
        }
        _CACHE[("static", SH)] = static

    # small weight-derived consts are ready at entry — put them before
    # the big arrays so they never sit on the critical-path tail
    wmap = make_weight_inputs({"N": x.shape[0], "NPAD": SH * N_CORES}, weights)
    wdev = {}
    for name, a in wmap.items():
        wdev[name] = put(name, np.ascontiguousarray(
            np.broadcast_to(a, (N_CORES,) + a.shape)
              .reshape(N_CORES * a.shape[0], a.shape[1])))

    # transfers of the big arrays start (async) as each one is built
    meta, dev = preprocess(x, edge_index, edge_weights, n_cores=N_CORES,
                           put=put)
    sharded, in_names, out_names, zero_shapes = _get_compiled(meta)

    assert SH == meta["SH"] and len(zero_shapes) == 1
    concat_in = [dev[n] if n in dev else static[n] if n in static else wdev[n]
                 for n in in_names]
    out_arrs = sharded(*concat_in, zeros_out)

    # output is replicated across cores by the on-device AllGather:
    # fetch a single shard (one D2H round-trip) instead of all eight
    out = np.asarray(
        out_arrs[out_names.index("out")].addressable_shards[0].data)
    out = out.reshape(N_CORES, C_OUT, SH).transpose(0, 2, 1).reshape(-1, C_OUT)
    return out[:meta["N"]]


# revision 57
# speedup vs baseline: 1.0002x; 1.0002x over previous
"""Self-contained Trainium2 Bass kernel for nn_GCN3 (3-layer GCN + BN + final linear).

Strategy: nodes/edges sharded by destination across 8 NeuronCores; edges
pre-sorted by destination and packed into 128-edge tiles. The full GCN
symmetric normalization (deg^-1/2 on both ends times edge weight) is baked
into a per-edge bf16 weight on the host. On device, per-tile gathers pull
source features from a bf16 node-feature table exchanged via AllGather;
destination one-hot matrices are built on the fly with a batched is_equal
against an iota, and a single matmul per tile accumulates [H, 128] block
outputs in PSUM. BatchNorm is folded into the next GEMM; BN statistics are
AllReduced. The compiled program and jitted runner are cached across calls.
"""
import sys
import numpy as np
import ml_dtypes

for _p in ("/opt/trn_rl_repo",):
    if _p not in sys.path:
        sys.path.insert(0, _p)

P = 128          # partitions / edges per tile
F_IN = 64
H = 32
C_OUT = 2
BN_EPS = 1e-5
N_CORES = 8
TBG = 32         # tiles per gather buffer
TBO = 16         # tiles per one-hot build
FCHUNK = 512     # final linear chunk

BF16 = ml_dtypes.bfloat16


def _to_bf16(a_f32):
    """Round-to-nearest-even f32 -> bf16 via integer ops (faster than astype)."""
    u = np.ascontiguousarray(a_f32).view(np.uint32)
    u16 = ((u + 0x7FFF + ((u >> 16) & 1)) >> 16).astype(np.uint16)
    return u16.view(BF16)


try:
    import numba as _nb

    @_nb.njit(cache=True, fastmath=False)
    def _edges_nb(row, col, w, dis, nxt, brow, tbase, gwv, dhiv):
        # per edge: counting-rank slot assignment + norm (deg^-1/2 w deg^-1/2)
        # + bf16 round-to-nearest-even + value packing + both scatters
        scr = np.empty(1, np.float32)
        scr_u = scr.view(np.uint32)
        for e in range(row.shape[0]):
            c = col[e]
            s_node = row[e]
            g = c >> 7
            r = nxt[g]
            nxt[g] = r + 1
            ri = brow[g] + (r & 127)
            ci = tbase[g] + (r >> 7)
            scr[0] = dis[s_node] * w[e] * dis[c]
            u = scr_u[0]
            b16 = (u + np.uint32(0x7FFF) + ((u >> np.uint32(16))
                                            & np.uint32(1))) >> np.uint32(16)
            gwv[ri, ci] = (b16 << np.uint32(16)) | (np.uint32(s_node)
                                                    & np.uint32(0xFFFF))
            dhiv[ri, ci] = np.uint8(((c & 127) << 1) | (s_node >> 16))

    @_nb.njit(cache=True, fastmath=False)
    def _xc_nb(xu16, xc, n_cores, SH, SH0, SH1, N):
        # truncating f32->bf16 transpose of node columns [SH0, SH1) of
        # every core into concat layout [n_cores*F, SH1-SH0]
        F = xu16.shape[1] // 2
        for c in range(n_cores):
            base_row = c * F
            n1 = min(c * SH + SH1, N)
            for n in range(c * SH + SH0, n1):
                cloc = n - c * SH - SH0
                for f in range(F):
                    xc[base_row + f, cloc] = xu16[n, 2 * f + 1]
except ImportError:          # pragma: no cover
    _nb = None


def preprocess(x, edge_index, edge_weights, n_cores=N_CORES, put=None,
               after_first_put=None):
    """Vectorized host-side tiling. Returns (meta, arrays).

    If `put` is given, each big array is handed to it as soon as it is
    built (used to overlap host->device transfer with preprocessing);
    arrays dict then holds whatever `put` returned. `after_first_put`
    fires once the first (largest) transfer is on the wire — a slot for
    cheap put dispatches that must not delay it.
    """
    N = x.shape[0]
    SH = -(-N // (n_cores * P)) * P               # nodes per core (padded)
    NPAD = SH * n_cores
    NBLK = SH // P
    out = {}
    keep = put if put is not None else (lambda name, a: a)

    # x first -> concat-layout bf16 (truncating cast: BN downstream
    # cancels the tiny systematic shrink), in two column-halves so the
    # first transfer hits the wire after only half the build
    xu16 = np.ascontiguousarray(np.asarray(x, np.float32)).view(np.uint16)
    SHH = (NBLK // 2) * P
    for name, s0, s1 in (("xTa", 0, SHH), ("xTb", SHH, SH)):
        xch = np.zeros((n_cores * F_IN, s1 - s0), np.uint16)
        if _nb is not None:
            _xc_nb(xu16, xch, n_cores, SH, s0, s1, N)
        else:
            for c in range(n_cores):
                lo = c * SH + s0
                hi = min(c * SH + s1, N)
                if hi > lo:
                    xch[c * F_IN:(c + 1) * F_IN, :hi - lo] = \
                        xu16[lo:hi, 1::2].T
        out[name] = keep(name, xch.view(BF16))
        if name == "xTa" and after_first_put is not None:
            after_first_put()

    row = np.asarray(edge_index[0], np.int32)
    col = np.asarray(edge_index[1], np.int32)
    w = np.asarray(edge_weights, np.float32)
    E = len(col)

    # symmetric normalization baked into per-edge weights
    deg = np.bincount(col, weights=w, minlength=N) + 1.0   # + self-loop
    dis = (1.0 / np.sqrt(deg)).astype(np.float32)

    NGB = NPAD // P
    # per-block self-loop count: loops of block g occupy ranks [0, nloop_g)
    nloop = np.minimum(np.maximum(N - np.arange(NGB) * P, 0), P)
    cnt = np.bincount(col >> 7, minlength=NGB) + nloop
    tiles_blk = np.maximum(-(-cnt.reshape(n_cores, NBLK) // P), 1).max(axis=0)
    NT = int(tiles_blk.sum())
    tile_off = np.zeros(NBLK + 1, np.int64)
    tile_off[1:] = np.cumsum(tiles_blk)

    base_g = ((np.arange(NGB, dtype=np.int32) // NBLK) * P * NT
              + np.tile(tile_off[:NBLK], n_cores)).astype(np.int32)

    brow_g = ((np.arange(NGB, dtype=np.int32) // NBLK) * P).astype(np.int32)
    tbase_g = np.tile(tile_off[:NBLK], n_cores).astype(np.int32)
    # one merged buffer: per partition-row, 4*NT bytes of gw then NT of dhi
    #   gw = low16(src) | bf16(norm) << 16 ; dhi = dloc << 1 | src >> 16
    buf = np.zeros((n_cores * P, 5 * NT), np.uint8)
    gwv = buf[:, :4 * NT].view(np.uint32)          # [n_cores*P, NT]
    dhiv = buf[:, 4 * NT:]                         # [n_cores*P, NT]
    # edge slots: counting ranks per block; self-loops occupy the first
    # nloop_g ranks
    if _nb is not None:
        _edges_nb(row, col, w, dis, nloop.astype(np.int32), brow_g,
                  tbase_g, gwv, dhiv)
    else:
        order = np.argsort((col >> 7).astype(np.int16), kind="stable")
        ch = cnt - nloop
        bstart = np.zeros(NGB, np.int32)
        np.cumsum(ch[:-1], out=bstart[1:], dtype=np.int32)
        r = np.arange(E, dtype=np.int32)
        r -= np.repeat((bstart - nloop).astype(np.int32), ch)
        ri = np.repeat(brow_g, ch) + (r & 127)
        ci = np.repeat(tbase_g, ch) + (r >> 7)
        rs = np.empty(E, np.int32); rs[order] = ri
        cs = np.empty(E, np.int32); cs[order] = ci
        norm_bf = _to_bf16(dis[row] * w * dis[col])
        val32 = norm_bf.view(np.uint16).astype(np.uint32)
        val32 <<= 16
        val32 |= (row & np.int32(0xFFFF)).astype(np.uint32)
        gwv[rs, cs] = val32
        dhiv[rs, cs] = ((col & 127) << 1 | (row >> 16)).astype(np.uint8)

    # self-loop slots: node v -> block g=v>>7, rank v&127 -> tile 0 of g
    v = np.arange(N, dtype=np.int32)
    gv = v >> 7
    rl = brow_g[gv] + (v & 127)
    cl = tbase_g[gv]
    lw = _to_bf16(dis * dis).view(np.uint16).astype(np.uint32) << 16
    lw |= v.astype(np.uint32) & 0xFFFF
    gwv[rl, cl] = lw
    dhiv[rl, cl] = ((v & 127) << 1 | (v >> 16)).astype(np.uint8)
    out["gwd"] = keep("gwd", buf)

    meta = dict(N=N, NPAD=NPAD, SH=SH, NBLK=NBLK, NT=NT,
                tiles_blk=tuple(int(t) for t in tiles_blk),
                tile_off=tile_off, n_cores=n_cores)
    return meta, out


import concourse.bass as bass
import concourse.bacc as bacc
import concourse.mybir as mybir
import concourse.tile as tile

F32 = mybir.dt.float32
MBF16 = mybir.dt.bfloat16
I32 = mybir.dt.int32
U8 = mybir.dt.uint8
AF = mybir.ActivationFunctionType


def build_program(meta):
    N = meta["N"]; NPAD = meta["NPAD"]; SH = meta["SH"]; NBLK = meta["NBLK"]
    NT = meta["NT"]
    tiles_blk = meta["tiles_blk"]; tile_off = meta["tile_off"]
    n_cores = meta["n_cores"]

    nc = bacc.Bacc()

    NBA = NBLK // 2
    SHH = NBA * P
    xTa_in = nc.declare_dram_parameter("xTa", [F_IN, SHH], MBF16,
                                       isOutput=False)
    xTb_in = nc.declare_dram_parameter("xTb", [F_IN, SH - SHH], MBF16,
                                       isOutput=False)
    gwd_in = nc.declare_dram_parameter("gwd", [P, 5 * NT], U8, isOutput=False)
    # output is AllGathered on-device so the host fetches ONE replica
    # (one D2H round-trip) instead of 8 per-core shards
    w1_in = nc.declare_dram_parameter("w1", [F_IN, H], MBF16, isOutput=False)
    cpack_in = nc.declare_dram_parameter("cpack", [H, 86], F32, isOutput=False)
    iota_in = nc.declare_dram_parameter("iota128", [P, P], MBF16, isOutput=False)
    ones_in = nc.declare_dram_parameter("ones_row", [1, SH], MBF16, isOutput=False)
    out_par = nc.declare_dram_parameter("out", [n_cores * C_OUT, SH], F32,
                                        isOutput=True)

    rg = [list(range(n_cores))]

    with tile.TileContext(nc) as tc:
        with (
            tc.tile_pool(name="cst", bufs=1) as cst,
            tc.tile_pool(name="big", bufs=1) as big,
            tc.tile_pool(name="st", bufs=2) as st,
            tc.tile_pool(name="gap", bufs=3) as gap,
            tc.tile_pool(name="ohp", bufs=3) as ohp,
            tc.tile_pool(name="wk", bufs=2) as wk,
            tc.tile_pool(name="psA", bufs=2, space="PSUM") as psA,
            tc.tile_pool(name="psB", bufs=4, space="PSUM") as psB,
            tc.tile_pool(name="dr", bufs=1, space="DRAM") as dr,
        ):
            # ---- consts ----
            w1_sb = cst.tile([F_IN, H], MBF16); nc.sync.dma_start(w1_sb[:], w1_in[:])
            cpack_sb = cst.tile([H, 86], F32)
            nc.sync.dma_start(cpack_sb[:], cpack_in[:])
            w23_sb = cpack_sb[:, 0:64]
            wl_sb = cpack_sb[:, 64:70]
            bl_sb = cpack_sb[0:C_OUT, 70:71]
            vec_sb = cpack_sb[:, 71:80]
            scorr_sb = cpack_sb[:, 80:86]
            iota_sb = cst.tile([P, P], MBF16); nc.sync.dma_start(iota_sb[:], iota_in[:])
            gw_sb = cst.tile([P, 4 * NT], U8)
            nc.sync.dma_start(gw_sb[:], gwd_in[:, 0:4 * NT])
            gw16 = gw_sb[:].bitcast(mybir.dt.uint16)       # [P, 2*NT]
            glo_ap = gw16[:, 0:2 * NT:2]                   # low halves
            dhi_t = cst.tile([P, NT], U8)
            nc.sync.dma_start(dhi_t[:], gwd_in[:, 4 * NT:5 * NT])
            dhi_u8 = dhi_t[:]
            wgt_sb = cst.tile([P, NT], MBF16)
            nc.vector.tensor_copy(wgt_sb[:], gw16[:, 1:2 * NT:2].bitcast(MBF16))
            # unpack: dloc = dhi >> 1 ; gidx = ((dhi & 1) << 16) + glo
            dhi_i = cst.tile([P, NT], I32)
            nc.vector.tensor_copy(dhi_i[:], dhi_u8)
            dloc_i = cst.tile([P, NT], I32)
            nc.vector.tensor_scalar(
                out=dloc_i[:], in0=dhi_i[:], scalar1=1, scalar2=None,
                op0=mybir.AluOpType.logical_shift_right)
            dloc_sb = cst.tile([P, NT], MBF16)
            nc.vector.tensor_copy(dloc_sb[:], dloc_i[:])
            gidx_sb = cst.tile([P, NT], I32)
            # hi<<16 = (dhi - (dloc<<1)) << 16 = (dhi<<16) - (dloc<<17)
            nc.vector.tensor_scalar(
                out=gidx_sb[:], in0=dhi_i[:], scalar1=16, scalar2=None,
                op0=mybir.AluOpType.logical_shift_left)
            nc.vector.tensor_scalar(
                out=dhi_i[:], in0=dloc_i[:], scalar1=17, scalar2=None,
                op0=mybir.AluOpType.logical_shift_left)
            nc.vector.tensor_tensor(out=gidx_sb[:], in0=gidx_sb[:], in1=dhi_i[:],
                                    op=mybir.AluOpType.subtract)
            glo_i = cst.tile([P, NT], I32)
            nc.vector.tensor_copy(glo_i[:], glo_ap)
            nc.vector.tensor_tensor(out=gidx_sb[:], in0=gidx_sb[:], in1=glo_i[:],
                                    op=mybir.AluOpType.add)

            # ---- slabs (relu outputs + ones row) ----
            slabs = []
            for k in range(3):
                s = big.tile([H + 1, SH], MBF16, tag=f"slab{k}")
                nc.sync.dma_start(s[H:H + 1, :], ones_in[:])
                slabs.append(s)
            hprime = big.tile([P, NBLK, H], MBF16, tag="hprime")

            oout_t = dr.tile([C_OUT, SH], F32, tag="oout")
            ofull_t = dr.tile([n_cores * C_OUT, SH], F32, tag="ofull")
            own_t = dr.tile([SH, H], MBF16, tag="own")
            table_t = dr.tile([NPAD, H], MBF16, tag="table")
            stat_in_t = dr.tile([H, 2], F32, tag="stat_in")
            stat_out_t = dr.tile([H, 2], F32, tag="stat_out")

            s_tiles, t_tiles = [], []

            for L in range(3):
                bvec = vec_sb[:, L:L + 1]
                gvec = vec_sb[:, 3 + L:4 + L]
                bevec = vec_sb[:, 6 + L:7 + L]

                # ---- GEMM -> h' (bf16) ----
                if L == 0:
                    for b in range(NBLK):
                        xblk = wk.tile([F_IN, P], MBF16, tag="xblk")
                        if b < NBA:
                            xsrc = xTa_in[:, b * P:(b + 1) * P]
                        else:
                            xsrc = xTb_in[:, (b - NBA) * P:(b - NBA + 1) * P]
                        nc.sync.dma_start(xblk[:], xsrc)
                        h_ps = psA.tile([P, H], F32, space="PSUM", tag="a")
                        nc.tensor.matmul(out=h_ps[:], lhsT=xblk[:], rhs=w1_sb[:],
                                         start=True, stop=True)
                        nc.vector.tensor_copy(hprime[:, b, :], h_ps[:])
                else:
                    s_prev, t_prev = s_tiles[-1], t_tiles[-1]
                    wsl = w23_sb[:, (L - 1) * H:L * H]
                    w_ext = wk.tile([H + 1, H], MBF16, tag="wext")
                    nc.vector.tensor_scalar_mul(w_ext[0:H, :], wsl, s_prev[:, :1])
                    br_ps = psB.tile([1, H], F32, space="PSUM", tag="b")
                    nc.tensor.matmul(out=br_ps[:], lhsT=t_prev[:], rhs=wsl,
                                     start=True, stop=True)
                    nc.vector.tensor_copy(w_ext[H:H + 1, :], br_ps[:])
                    for b in range(NBLK):
                        h_ps = psA.tile([P, H], F32, space="PSUM", tag="a")
                        nc.tensor.matmul(
                            out=h_ps[:], lhsT=slabs[L - 1][:, b * P:(b + 1) * P],
                            rhs=w_ext[:], start=True, stop=True)
                        nc.vector.tensor_copy(hprime[:, b, :], h_ps[:])

                # ---- exchange ----
                nc.sync.dma_start(
                    own_t.opt().rearrange("(b p) h -> p b h", p=P), hprime[:])
                nc.gpsimd.collective_compute(
                    "AllGather", mybir.AluOpType.bypass,
                    ins=[own_t.opt()], outs=[table_t.opt()], replica_groups=rg)

                # ---- aggregate ----
                stats_s = st.tile([H, NBLK], F32, tag="ss")
                stats_q = st.tile([H, NBLK], F32, tag="sq")
                sq_scr = st.tile([H, P], F32, tag="sqscr")
                cur_gb = cur_ob = -1
                gath = oh = None
                for b in range(NBLK):
                    T = int(tiles_blk[b])
                    out_ps = psB.tile([H, P], F32, space="PSUM", tag="b")
                    for tl in range(T):
                        t = int(tile_off[b]) + tl
                        gb_i, gb_s = divmod(t, TBG)
                        if gb_i != cur_gb:
                            cur_gb = gb_i
                            t0 = gb_i * TBG
                            tn = min(TBG, NT - t0)
                            gath = gap.tile([P, TBG, H], MBF16, tag="ga")
                            for tg in range(tn):
                                nc.gpsimd.indirect_dma_start(
                                    out=gath[:, tg, :], out_offset=None,
                                    in_=table_t.opt(),
                                    in_offset=bass.IndirectOffsetOnAxis(
                                        ap=gidx_sb[:, t0 + tg:t0 + tg + 1], axis=0))
                            nc.vector.tensor_tensor(
                                out=gath[:, :tn, :], in0=gath[:, :tn, :],
                                in1=wgt_sb[:, t0:t0 + tn].unsqueeze(2)
                                    .to_broadcast([P, tn, H]),
                                op=mybir.AluOpType.mult)
                        ob_i, ob_s = divmod(t, TBO)
                        if ob_i != cur_ob:
                            cur_ob = ob_i
                            o0 = ob_i * TBO
                            on = min(TBO, NT - o0)
                            oh = ohp.tile([P, TBO, P], MBF16, tag="oh")
                            nc.vector.tensor_tensor(
                                out=oh[:, :on, :],
                                in0=dloc_sb[:, o0:o0 + on].unsqueeze(2)
                                    .to_broadcast([P, on, P]),
                                in1=iota_sb[:].unsqueeze(1).to_broadcast([P, on, P]),
                                op=mybir.AluOpType.is_equal)
                        nc.tensor.matmul(
                            out=out_ps[:], lhsT=gath[:, gb_s, :], rhs=oh[:, ob_s, :],
                            start=(tl == 0), stop=(tl == T - 1))
                    # epilogue: bias, relu, stats
                    dst = slabs[L][0:H, b * P:(b + 1) * P]
                    nc.scalar.activation(dst, out_ps[:], AF.Relu, bias=bvec)
                    nc.vector.tensor_reduce(out=stats_s[:, b:b + 1], in_=dst,
                                            axis=mybir.AxisListType.X,
                                            op=mybir.AluOpType.add)
                    nc.scalar.activation(sq_scr[:], dst, AF.Square,
                                         accum_out=stats_q[:, b:b + 1])

                # ---- BN stats -> s, t ----
                st2 = st.tile([H, 2], F32, tag="st2")
                nc.vector.tensor_reduce(out=st2[:, 0:1], in_=stats_s[:],
                                        axis=mybir.AxisListType.X,
                                        op=mybir.AluOpType.add)
                nc.vector.tensor_reduce(out=st2[:, 1:2], in_=stats_q[:],
                                        axis=mybir.AxisListType.X,
                                        op=mybir.AluOpType.add)
                nc.sync.dma_start(stat_in_t[:], st2[:])
                nc.gpsimd.collective_compute(
                    "AllReduce", mybir.AluOpType.add,
                    ins=[stat_in_t.opt()], outs=[stat_out_t.opt()], replica_groups=rg)
                stg = st.tile([H, 2], F32, tag="stg")
                nc.sync.dma_start(stg[:], stat_out_t.opt())
                nc.vector.tensor_tensor(out=stg[:], in0=stg[:],
                                        in1=scorr_sb[:, 2 * L:2 * L + 2],
                                        op=mybir.AluOpType.subtract)
                nc.vector.tensor_scalar_mul(stg[:], stg[:], 1.0 / N)
                mu = stg[:, 0:1]
                s_t = st.tile([H, 1], F32, tag=f"s{L}")
                t_t = st.tile([H, 1], F32, tag=f"t{L}")
                var_t = st.tile([H, 1], F32, tag="var")
                nc.vector.tensor_tensor(out=var_t[:], in0=mu, in1=mu,
                                        op=mybir.AluOpType.mult)
                nc.vector.tensor_tensor(out=var_t[:], in0=stg[:, 1:2], in1=var_t[:],
                                        op=mybir.AluOpType.subtract)
                nc.vector.tensor_scalar_add(var_t[:], var_t[:], BN_EPS)
                nc.scalar.activation(var_t[:], var_t[:], AF.Sqrt)
                nc.vector.reciprocal(var_t[:], var_t[:])
                nc.vector.tensor_tensor(out=s_t[:], in0=gvec, in1=var_t[:],
                                        op=mybir.AluOpType.mult)
                nc.vector.tensor_tensor(out=t_t[:], in0=mu, in1=s_t[:],
                                        op=mybir.AluOpType.mult)
                nc.vector.tensor_tensor(out=t_t[:], in0=bevec, in1=t_t[:],
                                        op=mybir.AluOpType.subtract)
                s_tiles.append(s_t)
                t_tiles.append(t_t)

            # ---- final linear ----
            c2_ps = psB.tile([C_OUT, 1], F32, space="PSUM", tag="b")
            for k in range(3):
                nc.tensor.matmul(out=c2_ps[:], lhsT=wl_sb[:, 2 * k:2 * k + 2],
                                 rhs=t_tiles[k][:], start=(k == 0), stop=(k == 2))
            c2_sb = st.tile([C_OUT, 1], F32, tag="c2sb")
            nc.vector.tensor_tensor(out=c2_sb[:], in0=c2_ps[:], in1=bl_sb[:],
                                    op=mybir.AluOpType.add)
            wls = []
            for k in range(3):
                wsc = st.tile([H, C_OUT], MBF16, tag=f"wls{k}")
                nc.vector.tensor_scalar_mul(wsc[:], wl_sb[:, 2 * k:2 * k + 2],
                                            s_tiles[k][:, :1])
                wls.append(wsc)
            for ch0 in range(0, SH, FCHUNK):
                cw = min(FCHUNK, SH - ch0)
                f_ps = psB.tile([C_OUT, FCHUNK], F32, space="PSUM", tag="b")
                for k in range(3):
                    nc.tensor.matmul(out=f_ps[:, :cw], lhsT=wls[k][:],
                                     rhs=slabs[k][0:H, ch0:ch0 + cw],
                                     start=(k == 0), stop=(k == 2))
                f_sb = wk.tile([C_OUT, FCHUNK], F32, tag="fsb")
                nc.scalar.activation(f_sb[:, :cw], f_ps[:, :cw], AF.Identity,
                                     bias=c2_sb[:, :1])
                nc.sync.dma_start(oout_t.opt()[:, ch0:ch0 + cw], f_sb[:, :cw])
            nc.gpsimd.collective_compute(
                "AllGather", mybir.AluOpType.bypass,
                ins=[oout_t.opt()], outs=[ofull_t.opt()], replica_groups=rg)
            nc.sync.dma_start(out_par[:], ofull_t.opt())
    nc.compile()
    return nc


def make_weight_inputs(meta, weights, xscale=None):
    """Core-uniform constant arrays. W1 rows absorb the int8 x scales."""
    w1 = np.asarray(weights["W1"], np.float32)
    if xscale is not None:
        w1 = w1 * xscale[:, None]
    n_pad = meta["NPAD"] - meta["N"]
    b_relu = [np.maximum(np.asarray(weights[f"b{k}"], np.float32), 0.0)
              for k in (1, 2, 3)]
    vec = np.stack([np.asarray(weights[k], np.float32) for k in
                    ("b1", "b2", "b3", "g1", "g2", "g3", "be1", "be2", "be3")],
                   axis=1)
    cpack = np.zeros((H, 86), np.float32)
    cpack[:, 0:32] = np.asarray(weights["W2"], np.float32)
    cpack[:, 32:64] = np.asarray(weights["W3"], np.float32)
    cpack[:, 64:70] = (np.asarray(weights["Wl"], np.float32)
                       .reshape(3, H, C_OUT).transpose(1, 0, 2)
                       .reshape(H, 3 * C_OUT))
    cpack[0:C_OUT, 70] = np.asarray(weights["bl"], np.float32)
    cpack[:, 71:80] = vec
    cpack[:, 80:86] = np.concatenate(
        [np.stack([n_pad * br, n_pad * br ** 2], axis=1) for br in b_relu],
        axis=1)
    return {
        "w1": w1.astype(BF16),
        "cpack": cpack,
    }


# -------- cached compile + jitted runner --------
_CACHE = {}


def _get_compiled(meta):
    key = (meta["N"], meta["NT"], meta["tiles_blk"])
    hit = _CACHE.get(key)
    if hit is not None:
        return hit

    nc = build_program(meta)

    import jax
    from jax.sharding import Mesh, PartitionSpec
    from jax.experimental.shard_map import shard_map
    from concourse import bass2jax

    bass2jax.install_neuronx_cc_hook()
    partition_name = (nc.partition_id_tensor.name
                      if nc.partition_id_tensor else None)
    in_names, out_names, out_avals, zero_shapes = [], [], [], []
    for alloc in nc.m.functions[0].allocations:
        if not isinstance(alloc, mybir.MemoryLocationSet):
            continue
        name = alloc.memorylocations[0].name
        if alloc.kind == "ExternalInput":
            if name != partition_name:
                in_names.append(name)
        elif alloc.kind == "ExternalOutput":
            shape = tuple(alloc.tensor_shape)
            dtype = mybir.dt.np(alloc.dtype)
            out_names.append(name)
            out_avals.append(jax.core.ShapedArray(shape, dtype))
            zero_shapes.append((shape, dtype))
    n_params = len(in_names)
    all_names = list(in_names) + list(out_names)
    if partition_name is not None:
        all_names.append(partition_name)
    donate = tuple(range(n_params, n_params + len(out_names)))

    def _body(*args):
        operands = list(args)
        if partition_name is not None:
            operands.append(bass2jax.partition_id_tensor())
        outs = bass2jax._bass_exec_p.bind(
            *operands,
            out_avals=tuple(out_avals),
            in_names=tuple(all_names),
            out_names=tuple(out_names),
            lowering_input_output_aliases=(),
            sim_require_finite=False,
            sim_require_nnan=False,
            nc=nc,
        )
        return tuple(outs)

    devices = jax.devices()[:meta["n_cores"]]
    mesh = Mesh(np.asarray(devices), ("core",))
    in_specs = (PartitionSpec("core"),) * (n_params + len(out_names))
    out_specs = (PartitionSpec("core"),) * len(out_names)
    sharded = jax.jit(
        shard_map(_body, mesh=mesh, in_specs=in_specs, out_specs=out_specs,
                  check_rep=False),
        donate_argnums=donate, keep_unused=True)
    entry = (sharded, in_names, out_names, zero_shapes)
    _CACHE[key] = entry
    return entry


def kernel(**inputs):
    import jax
    from jax.sharding import Mesh, NamedSharding, PartitionSpec

    x = inputs["x"]
    edge_index = inputs["edge_index"]
    edge_weights = inputs["edge_weights"]
    weights = {k: np.asarray(inputs[k], np.float32) for k in (
        "W1", "b1", "g1", "be1", "W2", "b2", "g2", "be2",
        "W3", "b3", "g3", "be3", "Wl", "bl")}

    mesh = Mesh(np.asarray(jax.devices()[:N_CORES]), ("core",))
    sh = NamedSharding(mesh, PartitionSpec("core"))
    put = lambda name, a: jax.device_put(a, sh)

    # output zero-buffer (donated) made on device, and static consts; both
    # are known from shapes alone and overlap everything else
    SH = -(-x.shape[0] // (N_CORES * P)) * P
    # donated output buffer: recycle last call's output array (the program
    # overwrites every element, so contents are irrelevant — buffer reuse
    # only); first call materializes zeros on-device
    zeros_out = _CACHE.pop(("prev_out", SH), None)
    if zeros_out is None:
        zmk = _CACHE.get(("zmk", SH))
        if zmk is None:
            import jax.numpy as jnp
            zmk = jax.jit(
                lambda: jnp.zeros((N_CORES * N_CORES * C_OUT, SH),
                                  jnp.float32),
                out_shardings=sh)
            _CACHE[("zmk", SH)] = zmk
        zeros_out = zmk()
    static = _CACHE.get(("static", SH))
    if static is None:
        iota = np.tile(np.arange(P, dtype=np.float32), (P, 1)).astype(BF16)
        static = {
            "iota128": put("iota128", np.ascontiguousarray(
                np.broadcast_to(iota, (N_CORES,) + iota.shape)
                  .reshape(N_CORES * P, P))),
            "ones_row": put("ones_row", np.ones((N_CORES, SH), BF16)),
        }
        _CACHE[("static", SH)] = static

    # small weight-derived consts: built now, but their put dispatches
    # fire after the first big transfer is already on the wire
    wmap = make_weight_inputs({"N": x.shape[0], "NPAD": SH * N_CORES}, weights)
    wdev = {}

    def _put_consts():
        for name, a in wmap.items():
            wdev[name] = put(name, np.ascontiguousarray(
                np.broadcast_to(a, (N_CORES,) + a.shape)
                  .reshape(N_CORES * a.shape[0], a.shape[1])))

    # transfers of the big arrays start (async) as each one is built
    meta, dev = preprocess(x, edge_index, edge_weights, n_cores=N_CORES,
                           put=put, after_first_put=_put_consts)
    sharded, in_names, out_names, zero_shapes = _get_compiled(meta)

    assert SH == meta["SH"] and len(zero_shapes) == 1
    concat_in = [dev[n] if n in dev else static[n] if n in static else wdev[n]
                 for n in in_names]
    out_arrs = sharded(*concat_in, zeros_out)

    # output is replicated across cores by the on-device AllGather:
    # fetch a single shard (one D2H round-trip) instead of all eight
    out_dev = out_arrs[out_names.index("out")]
    out = np.asarray(out_dev.addressable_shards[0].data)
    _CACHE[("prev_out", SH)] = out_dev    # recycled as next call's donation
    out = out.reshape(N_CORES, C_OUT, SH).transpose(0, 2, 1).reshape(-1, C_OUT)
    return out[:meta["N"]]
